# revision 2
# baseline (speedup 1.0000x reference)
"""NeuralOpinionDynamics Trainium2 kernel (8-core SPMD, row-sharded).

out = omega*relu(A_norm @ Z W_D^T) + (1-omega)*softmax(sigmoid(s_i - s_j)) @ Z W_C^T
      + delta*(tanh(Z W1^T + b1) W2^T + b2),   s = Z @ w_V

Two algebraic restructurings vs the direct formulation:

1. Convection/attention is separable: with t = s/S in [-1,1], B_ij =
   exp(sigmoid(s_i - s_j)) is approximated by a degree-14 polynomial in
   (s_i - s_j) expanded binomially, B_ij ~= sum_{l,p} t_i^l M[l,p] t_j^p.
   Then numer_i = sum_j B_ij [zwc_j|1] = Phi_i . (M @ H) with moments
   H[p,:] = sum_j t_j^p [zwc_j|1]. The whole [N,N] attention collapses to
   64 accumulating [128,16]x[128,129] moment matmuls + 8 small matmuls.
   Softmax scale-invariance makes the overall scale of B irrelevant.

2. Diffusion uses A_norm = dinv_i * adj_ij * dinv_j with adj binary:
   adj ships as fp8 (0/1 exact, 1 byte -> 4x less HBM than fp32 A_norm)
   and multiplies a bf16 stationary y = dinv .* (Z W_D'^T) on the PE
   (mixed bf16 x fp8 matmul verified exact on HW).

Sharding: core c owns output rows [1024c, 1024(c+1)); A/adj symmetric, so
the column slice adj[:, rows_c] (host-swizzled to [128, 64*1024] j-tile
layout) doubles as the row slice. Z^T and small weights replicated; no
collectives. Per core: 64 j-tiles; dif^T accumulated [D, M] over j-tiles,
transposed on PE at the end; reaction MLP as in a plain 2-layer matmul.
"""

import sys

sys.path.insert(0, "/opt/trn_rl_repo")

from math import comb

import numpy as np

import concourse.bass as bass
import concourse.mybir as mybir
from concourse import bacc
from concourse.bass_utils import run_bass_kernel_spmd
from concourse.masks import make_identity
from concourse.tile import TileContext

N = 8192
D = 128
NCORES = 8
M = N // NCORES            # rows per core = 1024
JT = N // 128              # j-tiles = 64
IT = M // 128              # i-tiles per core = 8
SLAB = 4                   # j-tiles per adj DMA slab
NSLAB = JT // SLAB
DEG = 14                   # attention polynomial degree
PP = 16                    # padded power count (DEG+1 -> 16)

FP32 = mybir.dt.float32
BF16 = mybir.dt.bfloat16
FP8 = mybir.dt.float8e4
AF = mybir.ActivationFunctionType
ALU = mybir.AluOpType

LAST_RESULTS = None
LAST_IN_MAPS = None


def build_program(reps=1, hwloop=False):
    nc = bacc.Bacc("TRN2", target_bir_lowering=False, debug=False)

    adj_d = nc.dram_tensor("adj", [128, JT * M], FP8, kind="ExternalInput")
    zt_d = nc.dram_tensor("zt", [D, N], BF16, kind="ExternalInput")
    zi_d = nc.dram_tensor("zi", [D, M], BF16, kind="ExternalInput")
    rhsw_d = nc.dram_tensor("rhsw", [D, 258], BF16, kind="ExternalInput")
    dinvt_d = nc.dram_tensor("dinvt", [128, JT], FP32, kind="ExternalInput")
    disc_d = nc.dram_tensor("disc", [128, IT], FP32, kind="ExternalInput")
    mt_d = nc.dram_tensor("mt", [PP, PP], BF16, kind="ExternalInput")
    w1t_d = nc.dram_tensor("w1t", [D, D], BF16, kind="ExternalInput")
    b1_d = nc.dram_tensor("b1", [D, 1], FP32, kind="ExternalInput")
    w2t_d = nc.dram_tensor("w2t", [D, D], BF16, kind="ExternalInput")
    b2_d = nc.dram_tensor("b2", [1, D], BF16, kind="ExternalInput")
    out_d = nc.dram_tensor("out", [M, D], FP32, kind="ExternalOutput")

    with TileContext(nc) as tc:
        with (
            tc.tile_pool(name="persist", bufs=1) as pp,
            tc.tile_pool(name="adjp", bufs=3) as ap_pool,
            tc.tile_pool(name="small", bufs=2) as smp,
            tc.tile_pool(name="pzw", bufs=2, space="PSUM") as pzw,
            tc.tile_pool(name="pdif", bufs=1, space="PSUM") as pdif,
            tc.tile_pool(name="ph", bufs=1, space="PSUM") as ph,
            tc.tile_pool(name="pz", bufs=3, space="PSUM") as pz,
        ):
            # ---- persistent SBUF ----
            zt = pp.tile([D, N], BF16)
            zi = pp.tile([D, M], BF16)
            rhsw = pp.tile([D, 258], BF16)
            dinvt = pp.tile([128, JT], FP32)
            disc = pp.tile([128, IT], FP32)
            mt = pp.tile([PP, PP], BF16)
            w1t = pp.tile([D, D], BF16)
            b1 = pp.tile([D, 1], FP32)
            w2t = pp.tile([D, D], BF16)
            b2 = pp.tile([1, D], BF16)
            y_all = pp.tile([128, JT * 128], BF16)
            zwc1_all = pp.tile([128, JT * 129], BF16)
            t_all = pp.tile([128, JT], FP32)
            t_core = pp.tile([128, IT], FP32)
            psi_f = pp.tile([128, JT * PP], FP32)
            psi_b = pp.tile([128, JT * PP], BF16)
            psi_core = pp.tile([128, IT * PP], FP32)
            difrelu = pp.tile([D, M], FP32)
            rea_sb = pp.tile([128, M], FP32)
            ht = pp.tile([D, M], BF16)
            ones1 = pp.tile([1, 128], BF16)
            ident = pp.tile([128, 128], FP32)

            nc.sync.dma_start(zt[:], zt_d[:])
            nc.sync.dma_start(zi[:], zi_d[:])
            nc.sync.dma_start(rhsw[:], rhsw_d[:])
            nc.sync.dma_start(dinvt[:], dinvt_d[:])
            nc.sync.dma_start(disc[:], disc_d[:])
            nc.sync.dma_start(mt[:], mt_d[:])
            nc.sync.dma_start(w1t[:], w1t_d[:])
            nc.sync.dma_start(b1[:], b1_d[:])
            nc.sync.dma_start(w2t[:], w2t_d[:])
            nc.sync.dma_start(b2[:], b2_d[:])

            nc.vector.memset(ones1[:], 1.0)
            make_identity(nc, ident[:])
            # softmax-denominator ones column per j-tile block
            zwc1_v = zwc1_all[:].rearrange("p (t c) -> p t c", c=129)
            nc.vector.memset(zwc1_v[:, :, 128], 1.0)
            # power-basis pads: col 0 = 1.0, cols DEG+1..15 = 0
            nc.vector.memset(psi_f[:], 0.0)
            psi_v = psi_f[:].rearrange("p (t c) -> p t c", c=PP)
            nc.vector.memset(psi_v[:, :, 0], 1.0)
            nc.vector.memset(psi_core[:], 0.0)
            psc_v = psi_core[:].rearrange("p (t c) -> p t c", c=PP)
            nc.vector.memset(psc_v[:, :, 0], 1.0)

            # ---- PSUM accumulators (allocated once, cleared by start=True) ----
            ps_dif = [
                pdif.tile([128, 512], FP32, tag=f"dif{h}", name=f"ps_dif{h}")
                for h in range(2)
            ]
            ps_h = ph.tile([PP, 129], FP32)

            import contextlib
            rep_ctx = tc.For_i(0, reps, 1) if hwloop and reps > 1 else None
            for _rep in range(1 if hwloop else reps):
              with (rep_ctx if rep_ctx is not None else contextlib.nullcontext()):
                # ---- j-tile stream: zw matmul, y/zwc1/t extract, dif accum ----
                for g in range(NSLAB):
                    adjs = ap_pool.tile([128, SLAB * M], FP8, tag="adj")
                    nc.sync.dma_start(
                        adjs[:], adj_d[:, g * SLAB * M : (g + 1) * SLAB * M]
                    )
                    for q in range(SLAB):
                        jt = g * SLAB + q
                        zw = pzw.tile([128, 258], FP32, tag="zw")
                        nc.tensor.matmul(
                            zw[:], zt[:, jt * 128 : (jt + 1) * 128], rhsw[:],
                            start=True, stop=True, skip_group_check=True,
                        )
                        nc.vector.tensor_scalar(
                            y_all[:, jt * 128 : (jt + 1) * 128], zw[:, 0:128],
                            dinvt[:, jt : jt + 1], None, op0=ALU.mult,
                        )
                        nc.vector.tensor_copy(
                            zwc1_all[:, jt * 129 : jt * 129 + 128], zw[:, 128:256]
                        )
                        nc.vector.tensor_copy(t_all[:, jt : jt + 1], zw[:, 256:257])
                        for h in range(2):
                            nc.tensor.matmul(
                                ps_dif[h][:],
                                y_all[:, jt * 128 : (jt + 1) * 128],
                                adjs[:, q * M + h * 512 : q * M + (h + 1) * 512],
                                start=(jt == 0), stop=(jt == JT - 1),
                                skip_group_check=True,
                            )

                # ---- power basis for all j (batched over tiles) ----
                nc.vector.tensor_copy(psi_v[:, :, 1], t_all[:])
                for l in range(2, DEG + 1):
                    nc.vector.tensor_mul(psi_v[:, :, l], psi_v[:, :, l - 1], t_all[:])
                nc.vector.tensor_copy(psi_b[:], psi_f[:])

                # ---- moments H[p,:] = sum_j t_j^p [zwc_j | 1] ----
                for jt in range(JT):
                    nc.tensor.matmul(
                        ps_h[:],
                        psi_b[:, jt * PP : (jt + 1) * PP],
                        zwc1_all[:, jt * 129 : (jt + 1) * 129],
                        start=(jt == 0), stop=(jt == JT - 1),
                        skip_group_check=True,
                    )
                hsb = smp.tile([PP, 129], BF16, tag="hsb")
                nc.vector.tensor_copy(hsb[:], ps_h[:])
                gp = pz.tile([PP, 129], FP32, tag="mix", name="gp")
                nc.tensor.matmul(gp[:], mt[:], hsb[:], start=True, stop=True,
                                 skip_group_check=True)
                gsb = smp.tile([PP, 129], BF16, tag="gsb")
                nc.vector.tensor_copy(gsb[:], gp[:])

                # ---- t for this core's own rows (partition layout) ----
                for it in range(IT):
                    tps = pz.tile([128, 2], FP32, tag="mix", name="tps")
                    nc.tensor.matmul(
                        tps[:], zi[:, it * 128 : (it + 1) * 128], rhsw[:, 256:258],
                        start=True, stop=True, skip_group_check=True,
                    )
                    nc.vector.tensor_copy(t_core[:, it : it + 1], tps[:, 0:1])
                nc.vector.tensor_copy(psc_v[:, :, 1], t_core[:])
                for l in range(2, DEG + 1):
                    nc.vector.tensor_mul(
                        psc_v[:, :, l], psc_v[:, :, l - 1], t_core[:]
                    )

                # ---- reaction MLP ----
                for hh in range(2):
                    t1 = pz.tile([128, 512], FP32, tag="mix", name="t1")
                    nc.tensor.matmul(
                        t1[:], w1t[:], zi[:, hh * 512 : (hh + 1) * 512],
                        start=True, stop=True, skip_group_check=True,
                    )
                    nc.scalar.activation(
                        ht[:, hh * 512 : (hh + 1) * 512], t1[:], AF.Tanh,
                        bias=b1[:], scale=1.0,
                    )
                for half_i in range(2):
                    rea_ps = pz.tile([128, 512], FP32, tag="mix", name="rea_ps")
                    for q in range(4):
                        it = half_i * 4 + q
                        sl = slice(q * 128, (q + 1) * 128)
                        nc.tensor.matmul(
                            rea_ps[:, sl], ht[:, it * 128 : (it + 1) * 128], w2t[:],
                            start=(q == 0), stop=False, skip_group_check=True,
                        )
                        nc.tensor.matmul(
                            rea_ps[:, sl], ones1[:], b2[:],
                            start=False, stop=(q == 3), skip_group_check=True,
                        )
                    nc.vector.tensor_copy(
                        rea_sb[:, half_i * 512 : (half_i + 1) * 512], rea_ps[:]
                    )

                # ---- finish: relu(dif^T), per-i-tile combine, write out ----
                for h in range(2):
                    nc.vector.tensor_scalar(
                        difrelu[:, h * 512 : (h + 1) * 512], ps_dif[h][:],
                        0.0, None, op0=ALU.max,
                    )
                for it in range(IT):
                    phps = pz.tile([PP, 128], FP32, tag="mix", name="phps")
                    nc.tensor.transpose(
                        phps[:], psi_core[:, it * PP : (it + 1) * PP], ident[:]
                    )
                    pht = smp.tile([PP, 128], BF16, tag="pht")
                    nc.vector.tensor_copy(pht[:], phps[:])
                    nm = pz.tile([128, 129], FP32, tag="mix", name="nm")
                    nc.tensor.matmul(nm[:], pht[:], gsb[:], start=True, stop=True,
                                     skip_group_check=True)
                    rcp = smp.tile([128, 1], FP32, tag="rcp")
                    nc.vector.reciprocal(rcp[:], nm[:, 128:129])
                    con_t = smp.tile([128, 128], FP32, tag="con")
                    nc.vector.tensor_scalar(
                        con_t[:], nm[:, 0:128], rcp[:], None, op0=ALU.mult
                    )
                    dift = pz.tile([128, 128], FP32, tag="mix", name="dift")
                    nc.tensor.transpose(
                        dift[:], difrelu[:, it * 128 : (it + 1) * 128], ident[:]
                    )
                    o2 = smp.tile([128, 128], FP32, tag="o2")
                    nc.vector.tensor_scalar(
                        o2[:], dift[:], disc[:, it : it + 1], None, op0=ALU.mult
                    )
                    o3 = smp.tile([128, 128], FP32, tag="o3")
                    nc.vector.tensor_add(o3[:], o2[:], con_t[:])
                    o4 = smp.tile([128, 128], FP32, tag="o4")
                    nc.vector.tensor_add(
                        o4[:], o3[:], rea_sb[:, it * 128 : (it + 1) * 128]
                    )
                    nc.sync.dma_start(out_d[it * 128 : (it + 1) * 128, :], o4[:])

    nc.compile()
    return nc


def _sigmoid(x):
    return 1.0 / (1.0 + np.exp(-np.float64(x)))


def prep_inputs(Z, A_norm, W_D, W_C, w_V, W1, b1, W2, b2, omega_logit, delta_logit):
    import ml_dtypes

    bf16 = ml_dtypes.bfloat16
    f8 = mybir.dt.np(FP8)

    Z = np.asarray(Z, dtype=np.float32)
    A_norm = np.asarray(A_norm, dtype=np.float32)
    omega = _sigmoid(omega_logit)
    delta = _sigmoid(delta_logit)

    # attention polynomial: fit exp(sigmoid(x)) on x = s_i - s_j via
    # Chebyshev in u = (t_i - t_j)/2, t = s/S, then binomial expansion
    # into the separable coefficient matrix Mm[l, p].
    s = Z.astype(np.float64) @ np.asarray(w_V, np.float64)
    S = float(np.max(np.abs(s))) * 1.01
    cheb = np.polynomial.chebyshev.Chebyshev.interpolate(
        lambda u: np.exp(_sigmoid(2.0 * S * u)), DEG, domain=[-1, 1]
    )
    p = cheb.convert(kind=np.polynomial.Polynomial).coef
    Mm = np.zeros((PP, PP))
    for k in range(DEG + 1):
        for l in range(k + 1):
            Mm[l, k - l] += p[k] * (0.5 ** k) * comb(k, l) * ((-1.0) ** (k - l))
    mt_np = np.ascontiguousarray(Mm.T.astype(bf16))

    # binary adjacency + degree factors
    mask = A_norm != 0
    degc = mask.sum(axis=1)
    dinv = np.where(degc > 0, 1.0 / np.sqrt(np.maximum(degc, 1)), 0.0).astype(
        np.float32
    )
    adj8 = mask.astype(f8)
    dinvt = np.ascontiguousarray(dinv.reshape(JT, 128).T)

    zt = np.ascontiguousarray(Z.T).astype(bf16)                    # [D, N]
    rhsw = np.concatenate(
        [
            (omega * np.asarray(W_D, np.float64)).T,
            ((1.0 - omega) * np.asarray(W_C, np.float64)).T,
            np.asarray(w_V, np.float64).reshape(D, 1) / S,
            np.zeros((D, 1)),
        ],
        axis=1,
    ).astype(bf16)
    rhsw = np.ascontiguousarray(rhsw)
    w1t = np.ascontiguousarray(np.asarray(W1, np.float64).T.astype(bf16))
    b1c = np.ascontiguousarray(np.asarray(b1, np.float32).reshape(D, 1))
    w2t = np.ascontiguousarray((delta * np.asarray(W2, np.float64)).T.astype(bf16))
    b2r = np.ascontiguousarray(
        (delta * np.asarray(b2, np.float64)).reshape(1, D).astype(bf16)
    )

    shared = {
        "zt": zt, "rhsw": rhsw, "dinvt": dinvt, "mt": mt_np,
        "w1t": w1t, "b1": b1c, "w2t": w2t, "b2": b2r,
    }
    in_maps = []
    for c in range(NCORES):
        sl = slice(c * M, (c + 1) * M)
        # column slice, swizzled to j-tile layout [128, JT*M]
        adj_c = adj8[:, sl].reshape(JT, 128, M).transpose(1, 0, 2).reshape(
            128, JT * M
        )
        in_maps.append({
            **shared,
            "adj": np.ascontiguousarray(adj_c),
            "zi": np.ascontiguousarray(zt[:, sl]),
            "disc": np.ascontiguousarray(dinv[sl].reshape(IT, 128).T),
        })
    return in_maps


def kernel(Z, A_norm, W_D, W_C, w_V, W1, b1, W2, b2, omega_logit, delta_logit):
    global LAST_RESULTS, LAST_IN_MAPS
    in_maps = prep_inputs(
        Z, A_norm, W_D, W_C, w_V, W1, b1, W2, b2, omega_logit, delta_logit
    )
    LAST_IN_MAPS = in_maps
    nc = build_program()
    LAST_RESULTS = run_bass_kernel_spmd(nc, in_maps, list(range(NCORES)))
    return np.concatenate(
        [LAST_RESULTS.results[c]["out"] for c in range(NCORES)], axis=0
    )
